# revision 1
# baseline (speedup 1.0000x reference)
"""CuboidSelfAttention Trainium2 kernel.

Problem: x (2, 8, 112, 112, 256) fp32 -> LayerNorm -> cuboid reorder
(2x7x7 cuboids => 2048 independent cuboids of 98 tokens) -> 8-head self
attention within each cuboid -> out projection -> inverse reorder.

Sharding: 2048 cuboids split across 8 cores (256 each); weights replicated.

Per-core dataflow (feature-major):
  host: cuboid-reorder -> x_core (25088 tokens, 256) fp32 token-major.
  device, per macro-chunk of 64 cuboids (6272 tokens):
    phase 1: LN token-major (bn_stats/bn_aggr, batched rsqrt), normalize to
             bf16, PE-transpose 128x128 blocks -> xnT [256, 6272] bf16.
    phase 2: per group of 16 cuboids:
      A: qT/kT = Wq/Wk^T @ xnT (feature-major, PSUM->SBUF evac w/ bias),
         scores^T per head via row-packed K=32 matmuls (kT stationary,
         token dim padded to 128), exp on ACT (one op per cuboid),
         softmax sums via ones-column matmuls accumulated into one PSUM
         tile (row j = cuboid j's sums).
      recip of all 16 cuboids' sums in one DVE op -> DRAM scratch.
      B: v token-major (xnT stationary), AV via col-packed matmuls
         (oT feature-major), normalize-evac with broadcast recip (DMA),
         proj feature-major, bias-add evac, DMA out.
  host: transpose + inverse cuboid reorder.
"""

import numpy as np
import ml_dtypes

import concourse.bass as bass
import concourse.bacc as bacc
import concourse.mybir as mybir
import concourse.tile as tile
from concourse.bass_utils import run_bass_kernel_spmd

# ---------------- problem constants (hardcoded) ----------------
B, T, H, W, C = 2, 8, 112, 112, 256
HEADS = 8
DH = C // HEADS  # 32
CT, CH, CW = 2, 7, 7
CV = CT * CH * CW  # 98
NT, NH, NW = T // CT, H // CH, W // CW  # 4, 16, 16
NOC = NT * NH * NW  # 1024
NCUB = B * NOC  # 2048
NCORES = 8
CUB_PC = NCUB // NCORES  # 256 cuboids per core
TOK_PC = CUB_PC * CV  # 25088 tokens per core
EPS = 1e-6

# device tiling
MC_CUB = 64              # cuboids per macro chunk
N_MC = CUB_PC // MC_CUB  # 4
MC_TOK = MC_CUB * CV     # 6272
N_TT = MC_TOK // 128     # 49 token tiles per macro chunk
G16 = 16                 # cuboids per sums group
SUB = 4                  # cuboids per sub-chunk
SUB_TOK = SUB * CV       # 392

F32 = mybir.dt.float32
BF16 = mybir.dt.bfloat16

_prog_cache = {}


def _build_program(has_beta: bool, stage: int = 9):
    nc = bacc.Bacc("TRN2")

    x_d = nc.dram_tensor("x", [TOK_PC, C], F32, kind="ExternalInput")
    wq_d = nc.dram_tensor("wq", [128, 2, C], BF16, kind="ExternalInput")
    wk_d = nc.dram_tensor("wk", [128, 2, C], BF16, kind="ExternalInput")
    wv_d = nc.dram_tensor("wv", [128, 2, C], BF16, kind="ExternalInput")
    wp_d = nc.dram_tensor("wp", [128, 2, C], BF16, kind="ExternalInput")
    qkbias_d = nc.dram_tensor("qkbias", [128, 4], F32, kind="ExternalInput")
    pbias_d = nc.dram_tensor("pbias", [128, 2], F32, kind="ExternalInput")
    vbias_d = nc.dram_tensor("vbias", [1, C], BF16, kind="ExternalInput")
    onescol_d = nc.dram_tensor("onescol", [CV, 32 * 128], BF16, kind="ExternalInput")
    ident_d = nc.dram_tensor("ident", [128, 128], BF16, kind="ExternalInput")
    ones98_d = nc.dram_tensor("ones98", [1, CV], BF16, kind="ExternalInput")
    y_d = nc.dram_tensor("y", [C, TOK_PC], F32, kind="ExternalOutput")

    from contextlib import ExitStack

    with tile.TileContext(nc) as tc:
        with ExitStack() as ctx:
            ep = ctx.enter_context
            consts = ep(tc.tile_pool(name="consts", bufs=1))
            xin_p = ep(tc.tile_pool(name="xin", bufs=10))
            stats_p = ep(tc.tile_pool(name="stats", bufs=3))
            xn_p = ep(tc.tile_pool(name="xn", bufs=4))
            ps_a = ep(tc.tile_pool(name="ps_a", bufs=6, space="PSUM"))
            xnt_p = ep(tc.tile_pool(name="xnt", bufs=2))
            qt_p = ep(tc.tile_pool(name="qt", bufs=3))
            kt_p = ep(tc.tile_pool(name="kt", bufs=3))
            sc_ps = ep(tc.tile_pool(name="sc_ps", bufs=1, space="PSUM"))
            attn_p = ep(tc.tile_pool(name="attn", bufs=24))
            recip_p = ep(tc.tile_pool(name="recip", bufs=2))
            recip_d_p = ep(tc.tile_pool(name="recip_dram", bufs=2, space="DRAM"))
            rb_p = ep(tc.tile_pool(name="rb", bufs=3))
            v_p = ep(tc.tile_pool(name="vsb", bufs=3))
            on_p = ep(tc.tile_pool(name="on", bufs=3))
            out_p = ep(tc.tile_pool(name="osb", bufs=4))
            # ---- constants ----
            wq_sb = consts.tile([128, 2, C], BF16)
            wk_sb = consts.tile([128, 2, C], BF16)
            wv_sb = consts.tile([128, 2, C], BF16)
            wp_sb = consts.tile([128, 2, C], BF16)
            nc.sync.dma_start(out=wq_sb, in_=wq_d[:, :, :])
            nc.sync.dma_start(out=wk_sb, in_=wk_d[:, :, :])
            nc.sync.dma_start(out=wv_sb, in_=wv_d[:, :, :])
            nc.sync.dma_start(out=wp_sb, in_=wp_d[:, :, :])
            qkb_sb = consts.tile([128, 4], F32)
            nc.sync.dma_start(out=qkb_sb, in_=qkbias_d[:, :])
            pb_sb = consts.tile([128, 2], F32)
            nc.sync.dma_start(out=pb_sb, in_=pbias_d[:, :])
            onescol_sb = consts.tile([CV, 32 * 128], BF16)
            nc.sync.dma_start(out=onescol_sb, in_=onescol_d[:, :])
            ident_sb = consts.tile([128, 128], BF16)
            nc.sync.dma_start(out=ident_sb, in_=ident_d[:, :])
            vb_sb = consts.tile([1, C], BF16)
            nc.sync.dma_start(out=vb_sb, in_=vbias_d[:, :])
            ones98_sb = consts.tile([1, CV], BF16)
            nc.sync.dma_start(out=ones98_sb, in_=ones98_d[:, :])
            eps_sb = consts.tile([128, 1], F32)
            nc.vector.memset(eps_sb, EPS)

            for mc in range(N_MC):
                mc_tok0 = mc * MC_TOK
                # ---------------- phase 1: LN + transpose ----------------
                xnt = xnt_p.tile([128, 2, MC_TOK], BF16)
                for sb in range(7):  # 7 sub-batches x 7 token tiles
                    mv = stats_p.tile([128, 7, 2], F32)
                    rr = stats_p.tile([128, 7], F32)
                    xts = []
                    for i in range(7):
                        tt = sb * 7 + i
                        xt = xin_p.tile([128, C], F32)
                        nc.sync.dma_start(
                            out=xt,
                            in_=x_d[mc_tok0 + tt * 128 : mc_tok0 + (tt + 1) * 128, :],
                        )
                        st = stats_p.tile([128, 6], F32)
                        nc.vector.bn_stats(out=st, in_=xt)
                        nc.vector.bn_aggr(out=mv[:, i, :], in_=st)
                        xts.append(xt)
                    # rr = 1/sqrt(var + eps), batched over 7 tiles
                    nc.scalar.activation(
                        out=rr,
                        in_=mv[:, :, 1],
                        func=mybir.ActivationFunctionType.Sqrt,
                        bias=eps_sb,
                        scale=1.0,
                    )
                    nc.vector.reciprocal(out=rr, in_=rr)
                    for i in range(7):
                        tt = sb * 7 + i
                        xn = xn_p.tile([128, C], BF16)
                        nc.gpsimd.tensor_scalar(
                            out=xn,
                            in0=xts[i],
                            scalar1=mv[:, i, 0:1],
                            scalar2=rr[:, i : i + 1],
                            op0=mybir.AluOpType.subtract,
                            op1=mybir.AluOpType.mult,
                        )
                        tp = ps_a.tile([128, 2, 128], BF16, tag="gp", bufs=3)
                        nc.tensor.transpose(tp[:, 0, :], xn[:, 0:128], ident_sb)
                        nc.tensor.transpose(tp[:, 1, :], xn[:, 128:256], ident_sb)
                        if tt % 2 == 0:
                            nc.vector.tensor_copy(
                                out=xnt[:, :, tt * 128 : (tt + 1) * 128], in_=tp
                            )
                        else:
                            nc.scalar.copy(
                                out=xnt[:, :, tt * 128 : (tt + 1) * 128], in_=tp
                            )

                if stage == 1:
                    nc.gpsimd.dma_start(
                        out=y_d[0:128, mc_tok0 : mc_tok0 + MC_TOK], in_=xnt[:, 0, :]
                    )
                    nc.gpsimd.dma_start(
                        out=y_d[128:256, mc_tok0 : mc_tok0 + MC_TOK], in_=xnt[:, 1, :]
                    )
                    continue

                # ---------------- phase 2: attention ----------------
                for g in range(4):  # groups of 16 cuboids
                    sums_ps = None
                    if stage >= 4:
                        sums_ps = ps_a.tile(
                            [128, SUB_TOK], F32, tag="sums", bufs=1, name="sums_ps"
                        )
                    attns = []
                    for s in range(4):  # sub-chunks of 4 cuboids
                        t0 = g * G16 * CV + s * SUB_TOK  # within macro chunk
                        # --- q/k projections, feature-major ---
                        qt = qt_p.tile([128, 2, SUB_TOK], BF16)
                        kt = kt_p.tile([128, 2, SUB, 128], BF16)
                        nc.vector.memset(kt[:, :, :, CV:128], 0.0)
                        for which, w_sb, dst_q in ((0, wq_sb, True), (1, wk_sb, False)):
                            for mt in range(2):
                                ps = ps_a.tile([128, SUB_TOK], F32, tag="gp", bufs=3)
                                for ktile in range(2):
                                    nc.tensor.matmul(
                                        ps,
                                        lhsT=w_sb[:, ktile, mt * 128 : (mt + 1) * 128],
                                        rhs=xnt[:, ktile, t0 : t0 + SUB_TOK],
                                        start=(ktile == 0),
                                        stop=(ktile == 1),
                                    )
                                bias = qkb_sb[:, which * 2 + mt : which * 2 + mt + 1]
                                if dst_q:
                                    nc.vector.tensor_scalar(
                                        out=qt[:, mt, :],
                                        in0=ps,
                                        scalar1=bias,
                                        scalar2=None,
                                        op0=mybir.AluOpType.add,
                                    )
                                else:
                                    nc.vector.tensor_scalar(
                                        out=kt[:, mt, :, 0:CV],
                                        in0=ps.rearrange("p (c v) -> p c v", v=CV),
                                        scalar1=bias,
                                        scalar2=None,
                                        op0=mybir.AluOpType.add,
                                    )
                        if stage == 2:
                            nc.gpsimd.dma_start(
                                out=y_d[0:128, mc_tok0 + t0 : mc_tok0 + t0 + SUB_TOK],
                                in_=qt[:, 0, :],
                            )
                            nc.gpsimd.dma_start(
                                out=y_d[128:256, mc_tok0 + t0 : mc_tok0 + t0 + SUB_TOK],
                                in_=qt[:, 1, :],
                            )
                            continue
                        # --- scores + exp + sums, per cuboid ---
                        for ci in range(4):
                            c16 = s * 4 + ci  # cuboid index within group of 16
                            # bank = rg: concurrent row-group MMs must hit
                            # different PSUM banks (same-bank concurrent PE
                            # writes crash); same-rg MMs serialize on the PE
                            # row group, so sharing a bank across grp is safe.
                            scps = sc_ps.tile([128, 4, 2, 256], F32)
                            for grp in range(2):
                                for rg in range(4):
                                    nc.tensor.matmul(
                                        scps[:, rg, grp, 0:CV],
                                        lhsT=kt[rg * 32 : (rg + 1) * 32, grp, ci, :],
                                        rhs=qt[
                                            rg * 32 : (rg + 1) * 32,
                                            grp,
                                            ci * CV : (ci + 1) * CV,
                                        ],
                                        tile_position=(rg * 32, 0),
                                    )
                            attn = attn_p.tile([128, HEADS, CV], BF16)
                            nc.scalar.activation(
                                out=attn[0:CV, :, :].rearrange(
                                    "p (g h) v -> p h g v", g=2
                                ),
                                in_=scps[0:CV, :, :, 0:CV],
                                func=mybir.ActivationFunctionType.Exp,
                            )
                            attns.append(attn)
                            if stage == 3:
                                tok_c = mc_tok0 + t0 + ci * CV
                                nc.gpsimd.dma_start(
                                    out=y_d[0:CV, tok_c : tok_c + CV],
                                    in_=attn[0:CV, 0, :],
                                )
                                continue
                            # sums: row (grp*16 + c16) of sums_ps accumulates
                            for grp in range(2):
                                j = grp * 16 + c16
                                nc.tensor.matmul(
                                    sums_ps,
                                    lhsT=onescol_sb[:, j * 128 : (j + 1) * 128],
                                    rhs=attn[0:CV, grp * 4 : (grp + 1) * 4, :].rearrange(
                                        "p h v -> p (h v)"
                                    ),
                                    start=(c16 == 0 and grp == 0),
                                    stop=(c16 == 15 and grp == 1),
                                    skip_group_check=True,
                                )
                    if stage < 4:
                        continue
                    # --- reciprocal of all 16 cuboids' sums -> DRAM scratch ---
                    recip_sb = recip_p.tile([32, SUB_TOK], F32)
                    nc.vector.reciprocal(out=recip_sb, in_=sums_ps[0:32, :])
                    recip_dram = recip_d_p.tile([32, SUB_TOK], F32)
                    nc.sync.dma_start(out=recip_dram, in_=recip_sb)
                    if stage == 4:
                        g0 = mc_tok0 + g * G16 * CV
                        nc.sync.dma_start(
                            out=y_d[0:32, g0 : g0 + SUB_TOK], in_=recip_sb
                        )
                        continue

                    # --- B phase: v, AV, normalize, proj ---
                    for s in range(4):
                        t0 = g * G16 * CV + s * SUB_TOK
                        # broadcast recips: rb[p=(hh,d), grp, c, q]
                        # one DMA per (grp, hh): in dims [[0,32],[392,4],[1,98]]
                        # (partition-dim step-0 broadcast over d, c = dram rows)
                        rb = rb_p.tile([128, 2, SUB, CV], F32)
                        for grp in range(2):
                            for hh in range(4):
                                r0 = grp * 16 + s * 4
                                src = recip_dram[r0, :]
                                src_b = bass.AP(
                                    tensor=src.tensor,
                                    offset=src.offset + hh * CV,
                                    ap=[[0, 32], [SUB_TOK, SUB], [1, CV]],
                                )
                                nc.scalar.dma_start(
                                    out=rb[hh * 32 : (hh + 1) * 32, grp, :, :],
                                    in_=src_b,
                                )
                        if stage == 5:
                            nc.sync.dma_start(
                                out=y_d[0:128, mc_tok0 + t0 : mc_tok0 + t0 + SUB_TOK],
                                in_=rb[:, 0, :, :],
                            )
                            continue
                        # v projection, token-major (2 cuboids per PSUM bank)
                        vsb = v_p.tile([CV, SUB, C], BF16)
                        for vh in range(2):
                            vps = ps_a.tile([CV, 2, C], F32, tag="gp", bufs=3)
                            for cj in range(2):
                                ci = vh * 2 + cj
                                for ktile in range(2):
                                    nc.tensor.matmul(
                                        vps[:, cj, :],
                                        lhsT=xnt[
                                            :, ktile, t0 + ci * CV : t0 + (ci + 1) * CV
                                        ],
                                        rhs=wv_sb[:, ktile, :],
                                        start=(ktile == 0),
                                        stop=(ktile == 1) and not has_beta,
                                    )
                                if has_beta:
                                    nc.tensor.matmul(
                                        vps[:, cj, :],
                                        lhsT=ones98_sb,
                                        rhs=vb_sb,
                                        start=False,
                                        stop=True,
                                    )
                            nc.vector.tensor_copy(
                                out=vsb[:, vh * 2 : (vh + 1) * 2, :], in_=vps
                            )
                        if stage == 6:
                            nc.gpsimd.dma_start(
                                out=y_d[0:CV, mc_tok0 + t0 : mc_tok0 + t0 + SUB_TOK],
                                in_=vsb[:, :, 0:CV],
                            )
                            continue
                        # AV: col-packed, oT feature-major + normalize evac
                        ons = []
                        for grp in range(2):
                            otps = ps_a.tile([128, SUB, CV], F32, tag="gp", bufs=3)
                            for ci in range(4):
                                for cg in range(4):
                                    nc.tensor.matmul(
                                        otps[cg * 32 : (cg + 1) * 32, ci, :],
                                        lhsT=vsb[
                                            :, ci, grp * 128 + cg * 32 : grp * 128 + (cg + 1) * 32
                                        ],
                                        rhs=attns[s * 4 + ci][0:CV, grp * 4 + cg, :],
                                        tile_position=(0, cg * 32),
                                    )
                            on = on_p.tile([128, SUB, CV], BF16)
                            nc.vector.tensor_tensor(
                                out=on,
                                in0=otps,
                                in1=rb[:, grp, :, :],
                                op=mybir.AluOpType.mult,
                            )
                            ons.append(on)
                        if stage == 7:
                            nc.gpsimd.dma_start(
                                out=y_d[0:128, mc_tok0 + t0 : mc_tok0 + t0 + SUB_TOK],
                                in_=ons[0],
                            )
                            continue
                        # proj
                        for mt in range(2):
                            pps = ps_a.tile([128, SUB_TOK], F32, tag="gp", bufs=3)
                            for ktile in range(2):
                                nc.tensor.matmul(
                                    pps,
                                    lhsT=wp_sb[:, ktile, mt * 128 : (mt + 1) * 128],
                                    rhs=ons[ktile].rearrange("p c v -> p (c v)"),
                                    start=(ktile == 0),
                                    stop=(ktile == 1),
                                )
                            osb = out_p.tile([128, SUB_TOK], F32)
                            nc.scalar.activation(
                                out=osb,
                                in_=pps,
                                func=mybir.ActivationFunctionType.Identity,
                                bias=pb_sb[:, mt : mt + 1],
                                scale=1.0,
                            )
                            nc.scalar.dma_start(
                                out=y_d[
                                    mt * 128 : (mt + 1) * 128,
                                    mc_tok0 + t0 : mc_tok0 + t0 + SUB_TOK,
                                ],
                                in_=osb,
                            )
    nc.finalize()
    return nc


# ---------------- host-side helpers ----------------

def _cuboid_fwd(x):
    """(B, T, H, W, C) -> (NCUB, CV, C)"""
    xr = x.reshape(B, NT, CT, NH, CH, NW, CW, C)
    xr = xr.transpose(0, 1, 3, 5, 2, 4, 6, 7)
    return np.ascontiguousarray(xr.reshape(NCUB, CV, C))


def _cuboid_inv(o):
    """(NCUB, CV, C) -> (B, T, H, W, C)"""
    o = o.reshape(B, NT, NH, NW, CT, CH, CW, C)
    o = o.transpose(0, 1, 4, 2, 5, 3, 6, 7)
    return np.ascontiguousarray(o.reshape(B, T, H, W, C))


def _prep_consts(ln_scale, ln_bias, w_qkv, w_proj, b_proj):
    bf = ml_dtypes.bfloat16
    scale = np.float32(DH) ** np.float32(-0.5)
    wg = (w_qkv.astype(np.float32) * ln_scale.astype(np.float32)[:, None])
    wq = wg[:, 0:C] * scale
    wk = wg[:, C : 2 * C]
    wv = wg[:, 2 * C : 3 * C]
    qkv_bias = ln_bias.astype(np.float32) @ w_qkv.astype(np.float32)
    qb = qkv_bias[0:C] * scale
    kb = qkv_bias[C : 2 * C]
    vb = qkv_bias[2 * C : 3 * C]
    has_beta = bool(np.any(vb != 0.0) or np.any(qb != 0.0) or np.any(kb != 0.0))

    def ktiles(w):  # (256, 256) -> (128, 2, 256)
        return np.ascontiguousarray(
            w.reshape(2, 128, C).transpose(1, 0, 2)
        ).astype(bf)

    consts = {
        "wq": ktiles(wq),
        "wk": ktiles(wk),
        "wv": ktiles(wv),
        "wp": ktiles(w_proj.astype(np.float32)),
        "qkbias": np.ascontiguousarray(
            np.stack(
                [qb[0:128], qb[128:256], kb[0:128], kb[128:256]], axis=1
            )
        ).astype(np.float32),
        "pbias": np.ascontiguousarray(
            b_proj.astype(np.float32).reshape(2, 128).T
        ),
        "vbias": vb.reshape(1, C).astype(bf),
        "ones98": np.ones((1, CV), dtype=bf),
        "ident": np.eye(128, dtype=np.float32).astype(bf),
    }
    onescol = np.zeros((CV, 32, 128), dtype=np.float32)
    for j in range(32):
        onescol[:, j, j] = 1.0
    consts["onescol"] = onescol.reshape(CV, 32 * 128).astype(bf)
    return consts, has_beta


def _run(inputs, trace=False, tmpdir=None, stage=9):
    x = np.asarray(inputs["x"], dtype=np.float32)
    consts, has_beta = _prep_consts(
        np.asarray(inputs["ln_scale"], np.float32),
        np.asarray(inputs["ln_bias"], np.float32),
        np.asarray(inputs["w_qkv"], np.float32),
        np.asarray(inputs["w_proj"], np.float32),
        np.asarray(inputs["b_proj"], np.float32),
    )
    key = (has_beta, stage)
    if key not in _prog_cache:
        _prog_cache[key] = _build_program(has_beta, stage)
    nc = _prog_cache[key]

    xc = _cuboid_fwd(x)  # (2048, 98, 256)
    in_maps = []
    for core in range(NCORES):
        xcore = np.ascontiguousarray(
            xc[core * CUB_PC : (core + 1) * CUB_PC].reshape(TOK_PC, C)
        )
        m = {"x": xcore}
        m.update(consts)
        in_maps.append(m)

    res = run_bass_kernel_spmd(
        nc,
        in_maps,
        core_ids=list(range(NCORES)),
        trace=trace,
        tmpdir=tmpdir,
    )
    outs = []
    for core in range(NCORES):
        y = res.results[core]["y"]  # (256, 25088) feature-major
        outs.append(y.T.reshape(CUB_PC, CV, C))
    o = np.concatenate(outs, axis=0)
    return _cuboid_inv(o).astype(np.float32), res


def kernel(**inputs) -> np.ndarray:
    out, _ = _run(inputs, trace=False)
    return out



# revision 25
# speedup vs baseline: 1.5817x; 1.5817x over previous
"""CuboidSelfAttention Trainium2 kernel (v2).

Problem: x (2, 8, 112, 112, 256) fp32 -> LayerNorm -> cuboid reorder
(2x7x7 cuboids => 2048 independent cuboids of 98 tokens) -> 8-head self
attention within each cuboid -> out projection -> inverse reorder.

Sharding: 2048 cuboids split across 8 cores (256 each); weights replicated.

v2 design notes (from baseline trace analysis):
- LN normalize on DVE (gpsimd tensor_scalar measured ~4us/instr).
- rsqrt = Exp(-0.5*Ln(var+eps)) so the only ACT table set is
  natural_log_exp_and_others (exp/ln/identity/copy) -> no table thrash.
- softmax sums via 4-way col-packed one-hot matmuls into row
  32*ci + 2*s + g of one PSUM bank (was: 4x more PE cycles).
- scores: 4 row-strips concurrent; exp per head-pair so exp(pair0)
  overlaps scores(pair1) in different PSUM banks (sc_ps single tile,
  banks (pair,rg)).
- recip broadcast via DRAM roundtrip in bf16, one read DMA per
  (group, hh) with 784B runs; all DMAs issued from the sync queue.
- evacs alternate DVE/ACT by sub parity to balance engines.
- phase 1 (LN+transpose) of macro-chunk mc+1 is emitted interleaved
  with phase 2 groups of mc so engines overlap across phases.
- y staged in bf16 (halves output DMA); host converts to f32.
"""

import numpy as np
import ml_dtypes

import concourse.bass as bass
import concourse.bacc as bacc
import concourse.mybir as mybir
import concourse.tile as tile
from concourse.bass_utils import run_bass_kernel_spmd

# ---------------- problem constants (hardcoded) ----------------
B, T, H, W, C = 2, 8, 112, 112, 256
HEADS = 8
DH = C // HEADS  # 32
CT, CH, CW = 2, 7, 7
CV = CT * CH * CW  # 98
NT, NH, NW = T // CT, H // CH, W // CW  # 4, 16, 16
NOC = NT * NH * NW  # 1024
NCUB = B * NOC  # 2048
NCORES = 8
CUB_PC = NCUB // NCORES  # 256 cuboids per core
TOK_PC = CUB_PC * CV  # 25088 tokens per core
EPS = 1e-6

# device tiling
MC_CUB = 64              # cuboids per macro chunk
N_MC = CUB_PC // MC_CUB  # 4
MC_TOK = MC_CUB * CV     # 6272
N_TT = MC_TOK // 128     # 49 token tiles per macro chunk
G16 = 16                 # cuboids per sums group
SUB = 4                  # cuboids per sub-chunk
SUB_TOK = SUB * CV       # 392

F32 = mybir.dt.float32
BF16 = mybir.dt.bfloat16
AF = mybir.ActivationFunctionType

_prog_cache = {}

# Pin every ACT function to the natural_log_exp_and_others table set (it
# contains exp, ln, identity, copy -- everything this kernel uses) so the
# table-load pass emits exactly one load instead of thrashing between the
# exp-only and ln-only sets. Other sets are emptied, not removed, so
# act_func_set_id indices stay aligned with act_info.json.
_orig_gat = bacc.get_activation_tables


def _pinned_tables(arch):
    t = _orig_gat(arch)
    keep = "natural_log_exp_and_others"
    return {k: (v if k == keep else set()) for k, v in t.items()}


bacc.get_activation_tables = _pinned_tables


def _build_program(
    has_beta: bool, has_pbias: bool, interleave: bool = True, debug: bool = False
):
    nc = bacc.Bacc("TRN2")

    x_d = nc.dram_tensor("x", [TOK_PC, C], BF16, kind="ExternalInput")
    wq_d = nc.dram_tensor("wq", [128, 2, C], BF16, kind="ExternalInput")
    wk_d = nc.dram_tensor("wk", [128, 2, C], BF16, kind="ExternalInput")
    wv_d = nc.dram_tensor("wv", [128, 2, C], BF16, kind="ExternalInput")
    wp_d = nc.dram_tensor("wp", [128, 2, C], BF16, kind="ExternalInput")
    qkbias_d = nc.dram_tensor("qkbias", [128, 4], F32, kind="ExternalInput")
    pbias_d = nc.dram_tensor("pbias", [128, 2], F32, kind="ExternalInput")
    vbias_d = nc.dram_tensor("vbias", [1, C], BF16, kind="ExternalInput")
    onehot_d = nc.dram_tensor("onehot", [CV, 8, 32], BF16, kind="ExternalInput")
    ident_d = nc.dram_tensor("ident", [128, 128], BF16, kind="ExternalInput")
    ones98_d = nc.dram_tensor("ones98", [1, CV], BF16, kind="ExternalInput")
    y_d = nc.dram_tensor("y", [C, TOK_PC], BF16, kind="ExternalOutput")
    if debug:
        dbg_xnt = nc.dram_tensor("dbg_xnt", [C, MC_TOK], BF16, kind="ExternalOutput")
        dbg_qt = nc.dram_tensor("dbg_qt", [C, SUB_TOK], BF16, kind="ExternalOutput")
        dbg_kt = nc.dram_tensor("dbg_kt", [C, SUB, 128], BF16, kind="ExternalOutput")
        dbg_at = nc.dram_tensor("dbg_at", [128, HEADS, CV], BF16, kind="ExternalOutput")
        dbg_sm = nc.dram_tensor("dbg_sm", [128, SUB_TOK], F32, kind="ExternalOutput")
        dbg_rb = nc.dram_tensor("dbg_rb", [128, 8, SUB, CV], BF16, kind="ExternalOutput")
        dbg_on = nc.dram_tensor("dbg_on", [128, SUB, CV], BF16, kind="ExternalOutput")

    from contextlib import ExitStack

    with tile.TileContext(nc) as tc:
        with ExitStack() as ctx:
            ep = ctx.enter_context
            consts = ep(tc.tile_pool(name="consts", bufs=1))
            xin_p = ep(tc.tile_pool(name="xin", bufs=3))
            stats_p = ep(tc.tile_pool(name="stats", bufs=3))
            xn_p = ep(tc.tile_pool(name="xn", bufs=16))
            ps_a = ep(tc.tile_pool(name="ps_a", bufs=5, space="PSUM"))
            xnt_p = ep(tc.tile_pool(name="xnt", bufs=2))
            qt_p = ep(tc.tile_pool(name="qt", bufs=3))
            kt_p = ep(tc.tile_pool(name="kt", bufs=3))
            sc_ps = ep(tc.tile_pool(name="sc_ps", bufs=1, space="PSUM"))
            attn_p = ep(tc.tile_pool(name="attn", bufs=20))
            recip_p = ep(tc.tile_pool(name="recip", bufs=2))
            recip_d_p = ep(tc.tile_pool(name="recip_dram", bufs=2, space="DRAM"))
            rb_p = ep(tc.tile_pool(name="rb", bufs=2))
            v_p = ep(tc.tile_pool(name="vsb", bufs=3))
            on_p = ep(tc.tile_pool(name="on", bufs=4))
            out_p = ep(tc.tile_pool(name="osb", bufs=3))

            # ---- constants ----
            wq_sb = consts.tile([128, 2, C], BF16)
            wk_sb = consts.tile([128, 2, C], BF16)
            wv_sb = consts.tile([128, 2, C], BF16)
            wp_sb = consts.tile([128, 2, C], BF16)
            nc.sync.dma_start(out=wq_sb, in_=wq_d[:, :, :])
            nc.sync.dma_start(out=wk_sb, in_=wk_d[:, :, :])
            nc.sync.dma_start(out=wv_sb, in_=wv_d[:, :, :])
            nc.sync.dma_start(out=wp_sb, in_=wp_d[:, :, :])
            qkb_sb = consts.tile([128, 4], F32)
            nc.sync.dma_start(out=qkb_sb, in_=qkbias_d[:, :])
            pb_sb = consts.tile([128, 2], F32)
            nc.sync.dma_start(out=pb_sb, in_=pbias_d[:, :])
            oh_sb = consts.tile([CV, 8, 32], BF16)
            nc.sync.dma_start(out=oh_sb, in_=onehot_d[:, :, :])
            ident_sb = consts.tile([128, 128], BF16)
            nc.sync.dma_start(out=ident_sb, in_=ident_d[:, :])
            vb_sb = consts.tile([1, C], BF16)
            nc.sync.dma_start(out=vb_sb, in_=vbias_d[:, :])
            ones98_sb = consts.tile([1, CV], BF16)
            nc.sync.dma_start(out=ones98_sb, in_=ones98_d[:, :])
            eps_sb = consts.tile([128, 1], F32)
            nc.vector.memset(eps_sb, EPS)

            # xnt tiles per mc (rotated via pool bufs=2)
            xnt_tiles = {}

            def phase1_slice(mc, sbs):
                """LN + transpose for sub-batches `sbs` of macro chunk mc."""
                mc_tok0 = mc * MC_TOK
                if mc not in xnt_tiles:
                    xnt_tiles[mc] = xnt_p.tile(
                        [128, 2, MC_TOK], BF16, name="xnt"
                    )
                xnt = xnt_tiles[mc]
                for sb in sbs:
                    ntile = 7 if sb < 6 else 7  # 49 = 7*7
                    x7 = xin_p.tile([128, 7, C], BF16)
                    t0 = sb * 7 * 128
                    # one DMA for 7 token tiles (896 tokens)
                    src = x_d[mc_tok0 + t0, 0]
                    nc.sync.dma_start(
                        out=x7,
                        in_=bass.AP(
                            tensor=src.tensor,
                            offset=src.offset,
                            ap=[[C, 128], [128 * C, 7], [1, C]],
                        ),
                    )
                    mv = stats_p.tile([128, 7, 2], F32)
                    rr = stats_p.tile([128, 7], F32)
                    for i in range(7):
                        st = stats_p.tile([128, 6], F32, tag="st", bufs=3)
                        nc.vector.bn_stats(out=st, in_=x7[:, i, :])
                        nc.vector.bn_aggr(out=mv[:, i, :], in_=st)
                    # rr = exp(-0.5 * ln(var + eps))  (stays in exp/ln table)
                    nc.scalar.activation(
                        out=rr,
                        in_=mv[:, :, 1],
                        func=AF.Ln,
                        bias=eps_sb,
                        scale=1.0,
                    )
                    nc.scalar.activation(
                        out=rr, in_=rr, func=AF.Exp, bias=0.0, scale=-0.5
                    )
                    for i in range(7):
                        tt = sb * 7 + i
                        xn = xn_p.tile([128, C], BF16)
                        nc.vector.tensor_scalar(
                            out=xn,
                            in0=x7[:, i, :],
                            scalar1=mv[:, i, 0:1],
                            scalar2=rr[:, i : i + 1],
                            op0=mybir.AluOpType.subtract,
                            op1=mybir.AluOpType.mult,
                        )
                        tp = ps_a.tile([128, 2, 128], BF16, tag="gp", bufs=3)
                        nc.tensor.transpose(tp[:, 0, :], xn[:, 0:128], ident_sb)
                        nc.tensor.transpose(tp[:, 1, :], xn[:, 128:256], ident_sb)
                        if tt % 2 == 0:
                            nc.vector.tensor_copy(
                                out=xnt[:, :, tt * 128 : (tt + 1) * 128], in_=tp
                            )
                        else:
                            nc.scalar.copy(
                                out=xnt[:, :, tt * 128 : (tt + 1) * 128], in_=tp
                            )

            # phase-1 slices of the NEXT mc interleaved after each group
            P1_SLICES = [[0, 1], [2, 3], [4, 5], [6]]

            def dump(dst, src):
                nc.sync.dma_start(out=dst, in_=src)

            def phase2_group(mc, g):
                mc_tok0 = mc * MC_TOK
                xnt = xnt_tiles[mc]
                sums_ps = ps_a.tile(
                    [128, SUB_TOK], F32, tag="sums", bufs=1, name="sums_ps"
                )
                attns = []
                # ---------------- A phase ----------------
                for s in range(4):
                    t0 = g * G16 * CV + s * SUB_TOK
                    use_act = s % 2 == 1
                    dbg_this = debug and mc == 0 and g == 0 and s == 0
                    # --- q/k projections, feature-major ---
                    qt = qt_p.tile([128, 2, SUB_TOK], BF16)
                    kt = kt_p.tile([128, 2, SUB, 128], BF16)
                    for which, w_sb, dst_q in ((0, wq_sb, True), (1, wk_sb, False)):
                        for mt in range(2):
                            ps = ps_a.tile([128, SUB_TOK], F32, tag="gp", bufs=3)
                            for ktile in range(2):
                                nc.tensor.matmul(
                                    ps,
                                    lhsT=w_sb[:, ktile, mt * 128 : (mt + 1) * 128],
                                    rhs=xnt[:, ktile, t0 : t0 + SUB_TOK],
                                    start=(ktile == 0),
                                    stop=(ktile == 1),
                                )
                            if dst_q:
                                dst, src = qt[:, mt, :], ps
                            else:
                                dst = kt[:, mt, :, 0:CV]
                                src = ps.rearrange("p (c v) -> p c v", v=CV)
                            if has_beta:
                                bias = qkb_sb[
                                    :, which * 2 + mt : which * 2 + mt + 1
                                ]
                                nc.vector.tensor_scalar(
                                    out=dst,
                                    in0=src,
                                    scalar1=bias,
                                    scalar2=None,
                                    op0=mybir.AluOpType.add,
                                )
                            elif use_act and not dst_q:
                                nc.scalar.copy(out=dst, in_=src)
                            else:
                                nc.vector.tensor_copy(out=dst, in_=src)
                    if dbg_this:
                        for mt in range(2):
                            dump(dbg_qt[mt * 128 : (mt + 1) * 128, :], qt[:, mt, :])
                            dump(
                                dbg_kt[mt * 128 : (mt + 1) * 128, :, :],
                                kt[:, mt, :, :],
                            )
                    # --- scores + exp (per head pair) + sums ---
                    for ci in range(4):
                        # sc_ps banks: (pair, rg) -> 4 banks; grp within bank
                        scps = sc_ps.tile([128, 2, 2, 2, 256], F32)
                        attn = attn_p.tile([128, HEADS, CV], BF16)
                        attn_c = attn.rearrange("p (g c) v -> p c g v", g=2)
                        for pair in range(2):
                            for grp in range(2):
                                for rg in range(2):
                                    hg = pair * 2 + rg
                                    nc.tensor.matmul(
                                        scps[:, pair, rg, grp, 0:CV],
                                        lhsT=kt[
                                            hg * 32 : (hg + 1) * 32, grp, ci, :
                                        ],
                                        rhs=qt[
                                            hg * 32 : (hg + 1) * 32,
                                            grp,
                                            ci * CV : (ci + 1) * CV,
                                        ],
                                        tile_position=(hg * 32, 0),
                                    )
                            nc.scalar.activation(
                                out=attn_c[0:CV, pair * 2 : pair * 2 + 2, :, :],
                                in_=scps[0:CV, pair, :, :, 0:CV],
                                func=AF.Exp,
                            )
                        if debug and mc == 0 and g == 0 and s == 0 and ci == 0:
                            dump(dbg_at[:, :, :], attn)
                        attns.append(attn)
                        # sums: row 32*ci + 2*s + g of sums_ps
                        for grp in range(2):
                            j = 2 * s + grp
                            nc.tensor.matmul(
                                sums_ps[32 * ci : 32 * ci + 32, :],
                                lhsT=oh_sb[:, j, :],
                                rhs=attn[0:CV, grp * 4 : (grp + 1) * 4, :]
                                .rearrange("p h v -> p (h v)"),
                                start=(s == 0 and grp == 0),
                                stop=(s == 3 and grp == 1),
                                tile_position=(0, 32 * ci),
                                skip_group_check=True,
                            )
                # --- reciprocal -> DRAM scratch (layout: (2s+g)*1568 + hh*392
                #     + ci*98 + q, rows strided by partition = 32ci + 2s+g) ---
                if debug and mc == 0 and g == 0:
                    smsb = recip_p.tile([128, SUB_TOK], F32, tag="dbg", bufs=1)
                    nc.vector.tensor_copy(out=smsb, in_=sums_ps)
                    dump(dbg_sm[:, :], smsb)
                recip_sb = recip_p.tile([128, SUB_TOK], BF16)
                with nc.allow_low_precision(reason="softmax recip in bf16"):
                    nc.vector.reciprocal(out=recip_sb, in_=sums_ps)
                rdram = recip_d_p.tile([1, 8 * 4 * 4 * CV], BF16)
                rd0 = rdram[0, 0]
                for ci in range(4):
                    nc.scalar.dma_start(
                        out=bass.AP(
                            tensor=rd0.tensor,
                            offset=rd0.offset + ci * CV,
                            ap=[[4 * 4 * CV, 8], [4 * CV, 4], [1, CV]],
                        ),
                        in_=recip_sb[32 * ci : 32 * ci + 8, :],
                    )

                # ---------------- B phase ----------------
                # rb tile for the whole group: [128, (2s+g), ci, q]
                rb = rb_p.tile([128, 8, SUB, CV], BF16)
                for hh in range(4):
                    nc.scalar.dma_start(
                        out=rb[hh * 32 : (hh + 1) * 32, :, :, :],
                        in_=bass.AP(
                            tensor=rd0.tensor,
                            offset=rd0.offset + hh * 4 * CV,
                            ap=[[0, 32], [16 * CV, 8], [CV, 4], [1, CV]],
                        ),
                    )
                if debug and mc == 0 and g == 0:
                    dump(dbg_rb[:, :, :, :], rb)
                for s in range(4):
                    t0 = g * G16 * CV + s * SUB_TOK
                    use_act = s % 2 == 0
                    # v projection, token-major (2 cuboids per PSUM bank)
                    vsb = v_p.tile([CV, SUB, C], BF16)
                    for vh in range(2):
                        vps = ps_a.tile([CV, 2, C], F32, tag="gp", bufs=3)
                        for cj in range(2):
                            ci = vh * 2 + cj
                            for ktile in range(2):
                                nc.tensor.matmul(
                                    vps[:, cj, :],
                                    lhsT=xnt[
                                        :, ktile, t0 + ci * CV : t0 + (ci + 1) * CV
                                    ],
                                    rhs=wv_sb[:, ktile, :],
                                    start=(ktile == 0),
                                    stop=(ktile == 1) and not has_beta,
                                )
                            if has_beta:
                                nc.tensor.matmul(
                                    vps[:, cj, :],
                                    lhsT=ones98_sb,
                                    rhs=vb_sb,
                                    start=False,
                                    stop=True,
                                )
                        if use_act:
                            nc.scalar.copy(
                                out=vsb[:, vh * 2 : (vh + 1) * 2, :], in_=vps
                            )
                        else:
                            nc.vector.tensor_copy(
                                out=vsb[:, vh * 2 : (vh + 1) * 2, :], in_=vps
                            )
                    # AV: col-packed, oT feature-major + normalize evac
                    ons = []
                    for grp in range(2):
                        otps = ps_a.tile([128, SUB, CV], F32, tag="gp", bufs=3)
                        for ci in range(4):
                            for cg in range(4):
                                nc.tensor.matmul(
                                    otps[cg * 32 : (cg + 1) * 32, ci, :],
                                    lhsT=vsb[
                                        :,
                                        ci,
                                        grp * 128 + cg * 32 : grp * 128 + (cg + 1) * 32,
                                    ],
                                    rhs=attns[s * 4 + ci][0:CV, grp * 4 + cg, :],
                                    tile_position=(0, cg * 32),
                                )
                        on = on_p.tile([128, SUB, CV], BF16)
                        nc.vector.tensor_tensor(
                            out=on,
                            in0=otps,
                            in1=rb[:, 2 * s + grp, :, :],
                            op=mybir.AluOpType.mult,
                        )
                        if debug and mc == 0 and g == 0 and s == 0 and grp == 0:
                            dump(dbg_on[:, :, :], on)
                        ons.append(on)
                    # proj
                    osb = out_p.tile([128, 2, SUB_TOK], BF16)
                    for mt in range(2):
                        pps = ps_a.tile([128, SUB_TOK], F32, tag="gp", bufs=3)
                        for ktile in range(2):
                            nc.tensor.matmul(
                                pps,
                                lhsT=wp_sb[:, ktile, mt * 128 : (mt + 1) * 128],
                                rhs=ons[ktile].rearrange("p c v -> p (c v)"),
                                start=(ktile == 0),
                                stop=(ktile == 1),
                            )
                        if has_pbias:
                            nc.scalar.activation(
                                out=osb[:, mt, :],
                                in_=pps,
                                func=AF.Identity,
                                bias=pb_sb[:, mt : mt + 1],
                                scale=1.0,
                            )
                        elif (s + mt) % 2 == 0:
                            nc.scalar.copy(out=osb[:, mt, :], in_=pps)
                        else:
                            nc.vector.tensor_copy(out=osb[:, mt, :], in_=pps)
                    yy = y_d[0, mc_tok0 + t0]
                    nc.sync.dma_start(
                        out=bass.AP(
                            tensor=yy.tensor,
                            offset=yy.offset,
                            ap=[[TOK_PC, 128], [128 * TOK_PC, 2], [1, SUB_TOK]],
                        ),
                        in_=osb,
                    )

            # ---------------- main schedule ----------------
            phase1_slice(0, [0, 1, 2, 3, 4, 5, 6])
            if debug:
                xnt0 = xnt_tiles[0]
                dump(dbg_xnt[0:128, :], xnt0[:, 0, :])
                dump(dbg_xnt[128:256, :], xnt0[:, 1, :])
            for mc in range(N_MC):
                for g in range(4):
                    phase2_group(mc, g)
                    if interleave and mc + 1 < N_MC:
                        phase1_slice(mc + 1, P1_SLICES[g])
                if not interleave and mc + 1 < N_MC:
                    phase1_slice(mc + 1, [0, 1, 2, 3, 4, 5, 6])
    nc.finalize()
    return nc


# ---------------- host-side helpers ----------------

def _cuboid_fwd(x):
    """(B, T, H, W, C) -> (NCUB, CV, C)"""
    xr = x.reshape(B, NT, CT, NH, CH, NW, CW, C)
    xr = xr.transpose(0, 1, 3, 5, 2, 4, 6, 7)
    return np.ascontiguousarray(xr.reshape(NCUB, CV, C))


def _cuboid_inv(o):
    """(NCUB, CV, C) -> (B, T, H, W, C)"""
    o = o.reshape(B, NT, NH, NW, CT, CH, CW, C)
    o = o.transpose(0, 1, 4, 2, 5, 3, 6, 7)
    return np.ascontiguousarray(o.reshape(B, T, H, W, C))


def _prep_consts(ln_scale, ln_bias, w_qkv, w_proj, b_proj):
    bf = ml_dtypes.bfloat16
    scale = np.float32(DH) ** np.float32(-0.5)
    wg = (w_qkv.astype(np.float32) * ln_scale.astype(np.float32)[:, None])
    wq = wg[:, 0:C] * scale
    wk = wg[:, C : 2 * C]
    wv = wg[:, 2 * C : 3 * C]
    qkv_bias = ln_bias.astype(np.float32) @ w_qkv.astype(np.float32)
    qb = qkv_bias[0:C] * scale
    kb = qkv_bias[C : 2 * C]
    vb = qkv_bias[2 * C : 3 * C]
    has_beta = bool(np.any(vb != 0.0) or np.any(qb != 0.0) or np.any(kb != 0.0))
    has_pbias = bool(np.any(b_proj != 0.0))

    def ktiles(w):  # (256, 256) -> (128, 2, 256)
        return np.ascontiguousarray(
            w.reshape(2, 128, C).transpose(1, 0, 2)
        ).astype(bf)

    onehot = np.zeros((CV, 8, 32), dtype=np.float32)
    for j in range(8):
        onehot[:, j, j] = 1.0

    consts = {
        "wq": ktiles(wq),
        "wk": ktiles(wk),
        "wv": ktiles(wv),
        "wp": ktiles(w_proj.astype(np.float32)),
        "qkbias": np.ascontiguousarray(
            np.stack([qb[0:128], qb[128:256], kb[0:128], kb[128:256]], axis=1)
        ).astype(np.float32),
        "pbias": np.ascontiguousarray(
            b_proj.astype(np.float32).reshape(2, 128).T
        ),
        "vbias": vb.reshape(1, C).astype(bf),
        "ones98": np.ones((1, CV), dtype=bf),
        "ident": np.eye(128, dtype=np.float32).astype(bf),
        "onehot": onehot.astype(bf),
    }
    return consts, has_beta, has_pbias


def _run(inputs, trace=False, tmpdir=None, interleave=True, debug=False):
    x = np.asarray(inputs["x"], dtype=np.float32)
    consts, has_beta, has_pbias = _prep_consts(
        np.asarray(inputs["ln_scale"], np.float32),
        np.asarray(inputs["ln_bias"], np.float32),
        np.asarray(inputs["w_qkv"], np.float32),
        np.asarray(inputs["w_proj"], np.float32),
        np.asarray(inputs["b_proj"], np.float32),
    )
    key = (has_beta, has_pbias, interleave, debug)
    if key not in _prog_cache:
        _prog_cache[key] = _build_program(has_beta, has_pbias, interleave, debug)
    nc = _prog_cache[key]

    xc = _cuboid_fwd(x)  # (2048, 98, 256)
    in_maps = []
    for core in range(NCORES):
        xcore = np.ascontiguousarray(
            xc[core * CUB_PC : (core + 1) * CUB_PC].reshape(TOK_PC, C)
        ).astype(ml_dtypes.bfloat16)
        m = {"x": xcore}
        m.update(consts)
        in_maps.append(m)

    res = run_bass_kernel_spmd(
        nc,
        in_maps,
        core_ids=list(range(NCORES)),
        trace=trace,
        tmpdir=tmpdir,
    )
    outs = []
    for core in range(NCORES):
        y = res.results[core]["y"]  # (256, 25088) bf16 feature-major
        outs.append(y.astype(np.float32).T.reshape(CUB_PC, CV, C))
    o = np.concatenate(outs, axis=0)
    return _cuboid_inv(o).astype(np.float32), res


def kernel(**inputs) -> np.ndarray:
    out, _ = _run(inputs, trace=False)
    return out
